# revision 1
# baseline (speedup 1.0000x reference)
"""Trainium2 Bass kernel for a 5-layer bidirectional GRU (T=256, B=128, I=128, H=512, O=1).

Strategy:
  - Data-parallel over batch: 8 cores x 16 batch elements (SPMD, no collectives).
  - Everything feature-major [feature, token] so no transposes are ever needed:
      * recurrent matmul is weight-stationary: gh[3H, B] = sum_k WhhT[k].T @ h[k]
      * input-side gates gx[3H, tok] precomputed per 32-step block with big
        token-parallel matmuls (weights stationary across 512 token columns)
  - bf16 matmuls (fp32 is 4x slower on the PE), fp32 PSUM accumulate, fp32
    master hidden state, bf16 activations.
  - Both directions interleaved per step: two independent dependency chains
    keep the PE busy while the other direction's elementwise runs.
  - Activations ping-pong through internal DRAM between layers.
"""

import sys

sys.path.insert(0, "/opt/trn_rl_repo")

import numpy as np
import ml_dtypes

import concourse.bass as bass
import concourse.bacc as bacc
import concourse.mybir as mybir
import concourse.tile as tile
from concourse.vector_clock import ScopedClock, VectorClock
from concourse.bass_utils import run_bass_kernel_spmd

BF16 = mybir.dt.bfloat16
F32 = mybir.dt.float32
AF = mybir.ActivationFunctionType
OP = mybir.AluOpType

T, B, I, H, O, L = 256, 128, 128, 512, 1, 5
G3 = 3 * H  # 1536
NCORES = 8
BC = B // NCORES            # 16 batch per core
TOK = T * BC                # 4096 token columns per core
NBLK = T // 32              # 8 blocks of 32 timesteps
KH = H // 128               # 4 k-chunks of the hidden dim
M3 = G3 // 128              # 12 m-chunks of the gate dim
P = 128


class ChunkedDrainTC(tile.TileContext):
    """Work around walrus's 2-sync-wait limit on the kernel-tail drain by
    splitting the final drain into several drains with <=2 waits each."""

    def _drain_and_barrier(self, tick_clock, wait_clock):
        gc = tick_clock.global_clock
        n = len(gc)
        for i0 in range(0, n, 2):
            vec = [0] * n
            any_set = False
            for i in range(i0, min(i0 + 2, n)):
                vec[i] = gc[i]
                any_set = any_set or gc[i] > 0
            if not any_set:
                continue
            di = self.nc.sync.drain()
            wait_clock.add_sem_waits(di.ins, ScopedClock({None: VectorClock(vec)}))
        self.nc.all_engine_barrier()
        popped = self.nc._tile_sem_poison_stack.pop()
        assert popped is self._sem_poison
        self.nc.clear_and_free_semaphores(list(self.sems.allocated().values()))
        self.nc.all_engine_barrier()


def build_bass(t_steps=T, n_layers=L, repeat=1, elem_mode="full", whh_fp8=False):
    """Build the SPMD per-core program. Returns nc.

    repeat > 1 re-runs the whole network that many times (for timing by
    differencing out the fixed dispatch overhead)."""
    nblk = t_steps // 32
    tok = t_steps * BC
    WHH_DT = mybir.dt.float8e4 if whh_fp8 else BF16

    nc = bacc.Bacc(None)

    # ---- external I/O ----
    x_in = nc.dram_tensor("x", [1, P, tok], BF16, kind="ExternalInput")
    wih0_in = nc.dram_tensor("wih0", [2, I, G3], BF16, kind="ExternalInput")
    whh_in = nc.dram_tensor("whh", [n_layers, 2, H, G3], WHH_DT, kind="ExternalInput")
    gxb_in = nc.dram_tensor("gxb", [n_layers, 2, P, M3], F32, kind="ExternalInput")
    bhnb_in = nc.dram_tensor("bhnb", [n_layers, 2, 1, H], BF16, kind="ExternalInput")
    out_d = nc.dram_tensor("out", [1, tok], F32, kind="ExternalOutput")
    if n_layers > 1:
        wih_in = nc.dram_tensor(
            "wih", [n_layers - 1, 2, 2 * H, G3], BF16, kind="ExternalInput"
        )
    fcw_in = nc.dram_tensor("fcw", [2 * H, 1], BF16, kind="ExternalInput")
    fcb_in = nc.dram_tensor("fcb", [1, 1], F32, kind="ExternalInput")

    # ---- internal DRAM activation ping-pong [k-chunk, 128, tok] ----
    act_a = nc.dram_tensor("act_a", [2 * KH, P, tok], BF16)
    act_b = nc.dram_tensor("act_b", [2 * KH, P, tok], BF16)

    with ChunkedDrainTC(nc) as tc:
        with (
            tc.tile_pool(name="wpool", bufs=1) as wpool,        # weights
            tc.tile_pool(name="state", bufs=1) as state,        # per-layer state
            tc.tile_pool(name="stage", bufs=3) as stage_pool,   # act staging
            tc.tile_pool(name="tmp", bufs=4) as tmp_pool,       # elementwise temps
            tc.tile_pool(name="ghps", bufs=4, space="PSUM") as ghps_pool,
            tc.tile_pool(name="gxps", bufs=3, space="PSUM") as gxps_pool,
        ):
            for layer in [ly for _ in range(repeat) for ly in range(n_layers)]:
                ki = 1 if layer == 0 else 2 * KH  # input k-chunks
                act_in = x_in if layer == 0 else (act_a if layer % 2 == 1 else act_b)
                act_out = act_a if layer % 2 == 0 else act_b

                # ---- load weights/biases for both dirs ----
                whh_sb, wih_sb, gxb_sb, bhn_sb = [], [], [], []
                for d in range(2):
                    w = wpool.tile([P, KH, G3], WHH_DT, tag=f"whh{d}")
                    nc.sync.dma_start(
                        w[:], whh_in[layer, d].rearrange("(ko p) m -> p ko m", p=P)
                    )
                    whh_sb.append(w)
                    wi = wpool.tile([P, ki, G3], BF16, tag=f"wih{d}")
                    src = (
                        wih0_in[d]
                        if layer == 0
                        else wih_in[layer - 1, d]
                    ).rearrange("(ko p) m -> p ko m", p=P)
                    nc.sync.dma_start(wi[:], src)
                    wih_sb.append(wi)
                    gb = wpool.tile([P, M3], F32, tag=f"gxb{d}")
                    nc.sync.dma_start(gb[:], gxb_in[layer, d])
                    gxb_sb.append(gb)
                    bh = wpool.tile([1, H], BF16, tag=f"bhnb{d}")
                    nc.sync.dma_start(bh[:], bhnb_in[layer, d])
                    bhn_sb.append(bh)
                ones_sb = wpool.tile([1, BC], BF16, tag="ones")
                nc.vector.memset(ones_sb[:], 1.0)

                # ---- per-layer state ----
                h_hist = []     # bf16 hidden history ring [128, KH, 2, 32*BC]
                gx_ring = []    # bf16 input-gate ring  [128, M3, 2, 32*BC]
                for d in range(2):
                    hh = state.tile([P, KH, 2, 32 * BC], BF16, tag=f"hh{d}")
                    nc.vector.memset(hh[:], 0.0)
                    h_hist.append(hh)
                    gxr = state.tile([P, M3, 2, 32 * BC], BF16, tag=f"gx{d}", name=f"gx{d}")
                    gx_ring.append(gxr)

                stage_sb = [None, None]

                def emit_gx_stage(d, tb):
                    """DMA the act tokens of t-block tb into SBUF staging."""
                    st = stage_pool.tile([P, ki, 32 * BC], BF16, tag="stage", name="st")
                    nc.sync.dma_start(
                        st[:],
                        act_in[0:ki, :, tb * 32 * BC : (tb + 1) * 32 * BC].rearrange(
                            "k p c -> p k c"
                        ),
                    )
                    stage_sb[d] = st

                def emit_gx_group(d, tb, m):
                    """Input-side gate matmuls for one m-chunk of t-block tb."""
                    par = tb % 2
                    st = stage_sb[d]
                    ps = gxps_pool.tile([P, 32 * BC], F32, tag="gxps", name="gxps")
                    for k in range(ki):
                        nc.tensor.matmul(
                            ps[:],
                            wih_sb[d][:, k, m * P : (m + 1) * P],
                            st[:, k, :],
                            start=(k == 0),
                            stop=(k == ki - 1),
                        )
                    # copy psum -> ring with per-feature bias, f32 -> bf16
                    nc.scalar.activation(
                        gx_ring[d][:, m, par, :],
                        ps[:],
                        AF.Identity,
                        bias=gxb_sb[d][:, m : m + 1],
                    )

                def emit_flush(d, tb):
                    """Store finished hidden states of t-block tb to DRAM act."""
                    par = tb % 2
                    nc.sync.dma_start(
                        act_out[
                            d * KH : (d + 1) * KH, :, tb * 32 * BC : (tb + 1) * 32 * BC
                        ].rearrange("k p c -> p k c"),
                        h_hist[d][:, :, par, :],
                    )

                def emit_step(d, t, ghp):
                    """Recurrent matmuls for dir d into its half of the unified
                    PSUM tile ghp [128, 2, M3, BC]."""
                    td = t if d == 0 else (t_steps - 1 - t)  # token this step computes
                    prev = td - 1 if d == 0 else td + 1      # token holding h_{prev}
                    slp, pap = prev % 32, (prev // 32) % 2

                    base = d * M3 * BC
                    rhs = h_hist[d][:, :, pap, slp * BC : (slp + 1) * BC]
                    for m in range(M3):
                        has_bias_mm = m >= 8  # n chunks get b_hn via K=1 matmul
                        sl_ = slice(base + m * BC, base + (m + 1) * BC)
                        for k in range(KH):
                            nc.tensor.matmul(
                                ghp[:, sl_],
                                whh_sb[d][:, k, m * P : (m + 1) * P],
                                rhs[:, k, :],
                                start=(k == 0),
                                stop=(k == KH - 1) and not has_bias_mm,
                            )
                        if has_bias_mm:
                            nc.tensor.matmul(
                                ghp[:, sl_],
                                bhn_sb[d][0:1, (m - 8) * P : (m - 7) * P],
                                ones_sb[0:1, :],
                                start=False,
                                stop=True,
                            )

                def emit_elem_pair(t, ghp):
                    """Gate nonlinearity + state update, both dirs merged into
                    wide ops over the unified [128, 2, M3, BC] layout; only
                    gx-ring reads and h-slot writes are split per dir."""
                    ghv = ghp[:].rearrange("p (d m b) -> p d m b", d=2, b=BC)
                    ctx = []
                    for d in range(2):
                        td = t if d == 0 else (t_steps - 1 - t)
                        sl, pa = td % 32, (td // 32) % 2
                        prev = td - 1 if d == 0 else td + 1
                        slp, pap = prev % 32, (prev // 32) % 2
                        ctx.append((td, sl, pa, slp, pap))
                    if elem_mode == "dummy":
                        for d in range(2):
                            td, sl, pa, slp, pap = ctx[d]
                            nc.vector.tensor_copy(
                                h_hist[d][:, :, pa, sl * BC : (sl + 1) * BC],
                                ghv[:, d, 0:KH, :],
                            )
                        return

                    trz = tmp_pool.tile([P, 2, 8, BC], F32, tag="trz", name="trz")
                    for d in range(2):
                        td, sl, pa, slp, pap = ctx[d]
                        gx_rz = gx_ring[d][:, 0:8, pa, sl * BC : (sl + 1) * BC]
                        nc.vector.tensor_add(trz[:, d], ghv[:, d, 0:8, :], gx_rz)
                    rz = tmp_pool.tile([P, 2, 8, BC], F32, tag="rz", name="rz")
                    nc.scalar.activation(rz[:], trz[:], AF.Sigmoid)
                    tn = tmp_pool.tile([P, 2, KH, BC], F32, tag="tn", name="tn")
                    nc.vector.tensor_mul(tn[:], ghv[:, :, 8:12, :], rz[:, :, 0:KH, :])
                    for d in range(2):
                        td, sl, pa, slp, pap = ctx[d]
                        gx_n = gx_ring[d][:, 8:12, pa, sl * BC : (sl + 1) * BC]
                        nc.vector.tensor_add(tn[:, d], tn[:, d], gx_n)
                    n_t = tmp_pool.tile([P, 2, KH, BC], F32, tag="nt", name="nt")
                    nc.scalar.activation(n_t[:], tn[:], AF.Tanh)
                    dt_ = tmp_pool.tile([P, 2, KH, BC], F32, tag="dt", name="dt")
                    for d in range(2):
                        td, sl, pa, slp, pap = ctx[d]
                        h_prev = h_hist[d][:, :, pap, slp * BC : (slp + 1) * BC]
                        nc.vector.tensor_sub(dt_[:, d], h_prev, n_t[:, d])
                    nc.vector.tensor_mul(dt_[:], dt_[:], rz[:, :, KH : 2 * KH, :])
                    for d in range(2):
                        td, sl, pa, slp, pap = ctx[d]
                        nc.vector.tensor_add(
                            h_hist[d][:, :, pa, sl * BC : (sl + 1) * BC],
                            n_t[:, d],
                            dt_[:, d],
                        )

                # gx schedule: 24 m-groups (both dirs) spread over a block's 32
                # steps; group g of the NEXT block is emitted during step
                # ts where cumulative quota passes g.
                def gx_groups_due(ts):
                    lo = (ts * 24) // 32
                    hi = ((ts + 1) * 24) // 32
                    return range(lo, hi)

                # ---- prologue: stage + gx for the first consumed blocks ----
                emit_gx_stage(0, 0)
                emit_gx_stage(1, nblk - 1)
                for m in range(M3):
                    emit_gx_group(0, 0, m)
                    emit_gx_group(1, nblk - 1, m)
                for tb in range(nblk):
                    if tb < nblk - 1:
                        emit_gx_stage(0, tb + 1)
                        emit_gx_stage(1, nblk - 2 - tb)
                    for ts in range(32):
                        t = tb * 32 + ts
                        ghp = ghps_pool.tile(
                            [P, 2 * M3 * BC], F32, tag="ghps", name="ghps"
                        )
                        emit_step(0, t, ghp)
                        emit_step(1, t, ghp)
                        if tb < nblk - 1:
                            for g in gx_groups_due(ts):
                                d, m = g % 2, g // 2
                                emit_gx_group(d, tb + 1 if d == 0 else nblk - 2 - tb, m)
                        emit_elem_pair(t, ghp)
                    emit_flush(0, tb)
                    emit_flush(1, nblk - 1 - tb)

            # ---- final FC + sigmoid over the last layer's output ----
            act_fin = act_a if (n_layers - 1) % 2 == 0 else act_b
            fcw_sb = wpool.tile([P, 2 * KH, 1], BF16, tag="fcw")
            nc.sync.dma_start(fcw_sb[:], fcw_in.rearrange("(ko p) n -> p ko n", p=P))
            fcb_sb = wpool.tile([1, 1], F32, tag="fcb")
            nc.sync.dma_start(fcb_sb[:], fcb_in[:])
            out_sb = state.tile([1, tok], F32, tag="osb")
            for blk in range(nblk):
                st = stage_pool.tile([P, 2 * KH, 32 * BC], BF16, tag="stage")
                nc.sync.dma_start(
                    st[:],
                    act_fin[:, :, blk * 32 * BC : (blk + 1) * 32 * BC].rearrange(
                        "k p c -> p k c"
                    ),
                )
                ps = gxps_pool.tile([1, 32 * BC], F32, tag="gxps")
                for k in range(2 * KH):
                    nc.tensor.matmul(
                        ps[:],
                        fcw_sb[:, k, :],
                        st[:, k, :],
                        start=(k == 0),
                        stop=(k == 2 * KH - 1),
                    )
                nc.scalar.activation(
                    out_sb[:, blk * 32 * BC : (blk + 1) * 32 * BC],
                    ps[:],
                    AF.Sigmoid,
                    bias=fcb_sb[:, 0:1],
                )
            nc.sync.dma_start(out_d[:], out_sb[:])

    nc.finalize()
    return nc


def prep_inputs(input_seq, W_ih0, W_hh0, b_ih0, b_hh0, W_ih, W_hh, b_ih, b_hh,
                fc_w, fc_b, t_steps=T, n_layers=L, whh_fp8=False):
    """Host-side prep: transposes, bias folding, bf16 casts. Returns in_maps."""
    bf = ml_dtypes.bfloat16
    whh_dt = ml_dtypes.float8_e4m3fn if whh_fp8 else bf
    tok = t_steps * BC

    wih0 = np.ascontiguousarray(np.transpose(np.asarray(W_ih0), (0, 2, 1))).astype(bf)
    whh_all = np.concatenate(
        [np.asarray(W_hh0)[None], np.asarray(W_hh)], axis=0
    )[:n_layers]
    whh = np.ascontiguousarray(np.transpose(whh_all, (0, 1, 3, 2))).astype(whh_dt)
    bih_all = np.concatenate([np.asarray(b_ih0)[None], np.asarray(b_ih)], axis=0)[:n_layers]
    bhh_all = np.concatenate([np.asarray(b_hh0)[None], np.asarray(b_hh)], axis=0)[:n_layers]

    # gx bias: b_ih everywhere + b_hh on the r,z gates only (b_hn rides separately)
    gxb = bih_all.copy()
    gxb[:, :, : 2 * H] += bhh_all[:, :, : 2 * H]
    gxb = np.ascontiguousarray(
        np.transpose(gxb.reshape(n_layers, 2, M3, P), (0, 1, 3, 2))
    ).astype(np.float32)
    bhnb = np.ascontiguousarray(bhh_all[:, :, None, 2 * H :]).astype(bf)

    base = {
        "wih0": wih0,
        "whh": whh,
        "gxb": gxb,
        "bhnb": bhnb,
        "fcw": np.ascontiguousarray(np.asarray(fc_w).T).astype(bf),
        "fcb": np.asarray(fc_b, dtype=np.float32).reshape(1, 1),
    }
    if n_layers > 1:
        base["wih"] = np.ascontiguousarray(
            np.transpose(np.asarray(W_ih), (0, 1, 3, 2))
        )[: n_layers - 1].astype(bf)

    x = np.asarray(input_seq)[:t_steps]
    in_maps = []
    for c in range(NCORES):
        xc = x[:, c * BC : (c + 1) * BC, :].reshape(tok, I).T  # [128, tok]
        m = dict(base)
        m["x"] = np.ascontiguousarray(xc)[None].astype(bf)
        in_maps.append(m)
    return in_maps


def assemble_output(results, t_steps=T):
    """results: list of per-core dicts with 'out' [1, tok] -> [T, B, 1] f32."""
    outs = []
    for c in range(NCORES):
        o = np.asarray(results[c]["out"]).reshape(t_steps, BC)
        outs.append(o)
    return np.stack(outs, axis=1).reshape(t_steps, B)[:, :, None].astype(np.float32)


def kernel(**inputs):
    nc = build_bass()
    in_maps = prep_inputs(**inputs)
    res = run_bass_kernel_spmd(nc, in_maps, list(range(NCORES)))
    return assemble_output(res.results)



# revision 12
# speedup vs baseline: 12.8397x; 12.8397x over previous
"""Trainium2 Bass kernel for a 5-layer bidirectional GRU (T=256, B=128, I=128, H=512, O=1).

v2 strategy (direction-parallel):
  - 8 cores = 2 direction groups x 4 batch shards (BC=32).
    Cores 0-3 run the forward chains, cores 4-7 the backward chains on
    host-time-reversed input (identical SPMD program; direction is data).
  - Each layer ends with a pairwise AllGather (groups [c, c+4]) of the
    core's time-reversed activations; the next layer's input is
    [own activations (local, natural order), partner activations
    (gathered, already in own time order)]. Partner half is selected
    branch-free via (g0-g1)*sel + g1 with a per-core sel scalar.
  - Recurrent matmul is weight-stationary over 48 [128x128] tiles per
    step; the input-side gate tensor gx is pre-accumulated into the same
    PSUM via an identity matmul (also provides the bank clear), b_hn via
    K=1 bias matmuls. Elementwise chain is 7 fused ops (sigmoid, STT
    scale-mul, add, tanh, sub/mul/add) in bf16.
  - Whh optionally fp8 (x8192 host scale, descale folded into the
    activation scale / STT scalar), keeping LDWEIGHTS off the critical
    path. Input-side gx matmuls for the next block are interleaved into
    each step so the PE stays dense (HAM warm) during the elementwise
    window.
"""

import sys

sys.path.insert(0, "/opt/trn_rl_repo")

import numpy as np
import ml_dtypes

import concourse.bass as bass
import concourse.bacc as bacc
import concourse.mybir as mybir
import concourse.tile as tile
from concourse.vector_clock import ScopedClock, VectorClock
from concourse.bass_utils import run_bass_kernel_spmd

BF16 = mybir.dt.bfloat16
F32 = mybir.dt.float32
FP8 = mybir.dt.float8e4
AF = mybir.ActivationFunctionType
OP = mybir.AluOpType

T, B, I, H, O, L = 256, 128, 128, 512, 1, 5
G3 = 3 * H              # 1536
NCORES = 8
NSHARD = 4              # batch shards per direction group
BC = B // NSHARD        # 32 batch per core
TOK = T * BC            # 8192 token columns per core
NBLK = T // 32          # 8 blocks of 32 timesteps
BLKC = 32 * BC          # 1024 token columns per block
KH = H // 128           # 4 k-chunks of the hidden dim
M3 = G3 // 128          # 12 m-chunks of the gate dim
P = 128

WHH_FP8 = False
WSCALE = 8192.0 if WHH_FP8 else 1.0
WHH_DT = FP8 if WHH_FP8 else BF16


class ChunkedDrainTC(tile.TileContext):
    """Work around walrus's 2-sync-wait limit on the kernel-tail drain by
    splitting the final drain into several drains with <=2 waits each."""

    def _drain_and_barrier(self, tick_clock, wait_clock):
        gc = tick_clock.global_clock
        n = len(gc)
        for i0 in range(0, n, 2):
            vec = [0] * n
            any_set = False
            for i in range(i0, min(i0 + 2, n)):
                vec[i] = gc[i]
                any_set = any_set or gc[i] > 0
            if not any_set:
                continue
            di = self.nc.sync.drain()
            wait_clock.add_sem_waits(di.ins, ScopedClock({None: VectorClock(vec)}))
        self.nc.all_engine_barrier()
        popped = self.nc._tile_sem_poison_stack.pop()
        assert popped is self._sem_poison
        self.nc.clear_and_free_semaphores(list(self.sems.allocated().values()))
        self.nc.all_engine_barrier()


def build_bass(n_layers=L):
    nc = bacc.Bacc(None)

    # ---- external I/O (per core; direction already baked by the host) ----
    x_in = nc.dram_tensor("x", [1, P, TOK], BF16, kind="ExternalInput")
    wih0_in = nc.dram_tensor("wih0", [I, G3], BF16, kind="ExternalInput")
    whh_in = nc.dram_tensor("whh", [n_layers, H, G3], WHH_DT, kind="ExternalInput")
    gxb_in = nc.dram_tensor("gxb", [n_layers, P, M3], F32, kind="ExternalInput")
    bhnb_in = nc.dram_tensor("bhnb", [n_layers, KH, P], BF16, kind="ExternalInput")
    bsel_in = nc.dram_tensor("bsel", [KH, KH * BC], BF16, kind="ExternalInput")
    ident_in = nc.dram_tensor("ident", [P, P], BF16, kind="ExternalInput")
    sel_in = nc.dram_tensor("sel", [P, 1], F32, kind="ExternalInput")
    out_d = nc.dram_tensor("out", [1, TOK], F32, kind="ExternalOutput")
    if n_layers > 1:
        wih_in = nc.dram_tensor(
            "wih", [n_layers - 1, 2 * H, G3], BF16, kind="ExternalInput"
        )
    fcw_in = nc.dram_tensor("fcw", [2 * H, 1], BF16, kind="ExternalInput")
    fcb_in = nc.dram_tensor("fcb", [1, 1], F32, kind="ExternalInput")

    # ---- internal DRAM: own-activation ping-pong [k, 128, tok] ----
    act_a = nc.dram_tensor("act_a", [KH, P, TOK], BF16)
    act_b = nc.dram_tensor("act_b", [KH, P, TOK], BF16)

    with ChunkedDrainTC(nc) as tc:
        with (
            tc.tile_pool(name="const", bufs=1) as cpool,
            tc.tile_pool(name="wpool", bufs=1) as wpool,
            tc.tile_pool(name="state", bufs=1) as state,
            tc.tile_pool(name="stage", bufs=2) as stage_pool,
            tc.tile_pool(name="pair", bufs=2) as pair_pool,
            tc.tile_pool(name="tmp", bufs=4) as tmp_pool,
            tc.tile_pool(name="ghA", bufs=3, space="PSUM") as ghA_pool,
            tc.tile_pool(name="ghB", bufs=2, space="PSUM") as ghB_pool,
            tc.tile_pool(name="gxps", bufs=3, space="PSUM") as gxps_pool,
            tc.tile_pool(name="dram", bufs=2, space="DRAM") as dram_pool,
        ):
            ident_sb = cpool.tile([P, P], BF16, tag="ident")
            nc.sync.dma_start(ident_sb[:], ident_in[:])
            sel_sb = cpool.tile([P, 1], F32, tag="sel")
            nc.sync.dma_start(sel_sb[:], sel_in[:])
            bsel_sb = cpool.tile([KH, KH * BC], BF16, tag="bsel")
            nc.sync.dma_start(bsel_sb[:], bsel_in[:])

            gathered = None  # previous layer's AllGather result

            for layer in range(n_layers):
                ki = 1 if layer == 0 else 2 * KH
                act_own_prev = act_a if layer % 2 == 1 else act_b
                act_own = act_a if layer % 2 == 0 else act_b

                # ---- weights/biases for this layer ----
                whh_sb = wpool.tile([P, KH, G3], WHH_DT, tag="whh")
                nc.sync.dma_start(
                    whh_sb[:], whh_in[layer].rearrange("(ko p) m -> p ko m", p=P)
                )
                wih_sb = wpool.tile([P, ki, G3], BF16, tag="wih")
                src = (wih0_in[:] if layer == 0 else wih_in[layer - 1]).rearrange(
                    "(ko p) m -> p ko m", p=P
                )
                nc.sync.dma_start(wih_sb[:], src)
                gxb_sb = wpool.tile([P, M3], F32, tag="gxb")
                nc.sync.dma_start(gxb_sb[:], gxb_in[layer])
                bhn_sb = wpool.tile([KH, P], BF16, tag="bhn")
                nc.sync.dma_start(bhn_sb[:], bhnb_in[layer])

                # ---- per-layer state ----
                h_hist = state.tile([P, KH, 2, BLKC], BF16, tag="hh")
                nc.vector.memset(h_hist[:], 0.0)
                gx_ring = state.tile([P, M3, 2, BLKC], BF16, tag="gxr", name="gxr")

                # collective buffers for THIS layer's output exchange
                rev_t = dram_pool.tile([KH, P, TOK], BF16, tag="rev")
                gath_t = dram_pool.tile([2, KH, P, TOK], BF16, tag="gath")

                stage_sb = [None]

                def emit_stage(tb):
                    """Stage block tb of this layer's input into SBUF."""
                    st = stage_pool.tile([P, ki, BLKC], BF16, tag="stage", name="st")
                    c0, c1 = tb * BLKC, (tb + 1) * BLKC
                    if layer == 0:
                        nc.sync.dma_start(
                            st[:, 0, :], x_in[0, :, c0:c1]
                        )
                    else:
                        nc.sync.dma_start(
                            st[:, 0:KH, :],
                            act_own_prev[:, :, c0:c1].rearrange("k p c -> p k c"),
                        )
                        pr = pair_pool.tile([P, 2, KH, BLKC], BF16, tag="pr", name="pr")
                        nc.sync.dma_start(
                            pr[:],
                            gathered[:, :, :, c0:c1].rearrange("j k p c -> p j k c"),
                        )
                        dt_ = pair_pool.tile([P, KH, BLKC], BF16, tag="dsel", name="ds")
                        nc.vector.tensor_sub(dt_[:], pr[:, 0], pr[:, 1])
                        nc.vector.scalar_tensor_tensor(
                            st[:, KH : 2 * KH, :], dt_[:], sel_sb[:, 0:1], pr[:, 1],
                            OP.mult, OP.add,
                        )
                    stage_sb[0] = st

                def emit_gx_group(tb, g):
                    """Input-side gate matmuls for group g (m, half) of block tb."""
                    m, half = g // 2, g % 2
                    par = tb % 2
                    st = stage_sb[0]
                    ps = gxps_pool.tile([P, BLKC // 2], F32, tag="gxps", name="gxps")
                    for k in range(ki):
                        nc.tensor.matmul(
                            ps[:],
                            wih_sb[:, k, m * P : (m + 1) * P],
                            st[:, k, half * (BLKC // 2) : (half + 1) * (BLKC // 2)],
                            start=(k == 0),
                            stop=(k == ki - 1),
                        )
                    # rz chunks are pre-scaled by WSCALE so they share the PSUM
                    # scale of the fp8 recurrent matmul; n chunks stay unscaled.
                    sc = WSCALE if m < 8 else 1.0
                    nc.scalar.activation(
                        gx_ring[:, m, par, half * (BLKC // 2) : (half + 1) * (BLKC // 2)],
                        ps[:],
                        AF.Identity,
                        bias=gxb_sb[:, m : m + 1],
                        scale=sc,
                    )

                def emit_flush(tb):
                    """Store finished hidden states of block tb to DRAM (natural
                    order for own next-layer use, time-reversed for the
                    partner exchange)."""
                    par = tb % 2
                    c0, c1 = tb * BLKC, (tb + 1) * BLKC
                    nc.sync.dma_start(
                        act_own[:, :, c0:c1].rearrange("k p c -> p k c"),
                        h_hist[:, :, par, :],
                    )
                    rb = NBLK - 1 - tb
                    for k in range(KH):
                        dst = rev_t[k, :, rb * BLKC : (rb + 1) * BLKC].rearrange(
                            "p (ts b) -> p ts b", ts=32
                        )[:, ::-1, :]
                        nc.sync.dma_start(
                            dst,
                            h_hist[:, k, par, :].rearrange(
                                "p (ts b) -> p ts b", ts=32
                            ),
                        )

                def gx_groups_due(ts):
                    lo = (ts * 24) // 32
                    hi = ((ts + 1) * 24) // 32
                    return range(lo, hi)

                # ---- prologue: stage + gx for block 0 ----
                emit_stage(0)
                for g in range(24):
                    emit_gx_group(0, g)

                for tb in range(NBLK):
                    par = tb % 2
                    if tb < NBLK - 1:
                        emit_stage(tb + 1)
                    for ts in range(32):
                        t = tb * 32 + ts
                        prev = t - 1
                        slp, pap = prev % 32, (prev // 32) % 2

                        ghA = ghA_pool.tile([P, 8, BC], F32, tag="ghA", name="ghA")
                        ghB = ghB_pool.tile([P, KH, BC], F32, tag="ghB", name="ghB")
                        h_prev = h_hist[:, :, pap, slp * BC : (slp + 1) * BC]

                        # gx(rz) pre-load + bank clear
                        nc.tensor.matmul(
                            ghA[:].rearrange("p m b -> p (m b)"),
                            ident_sb[:],
                            gx_ring[:, 0:8, par, ts * BC : (ts + 1) * BC],
                            start=True,
                            stop=False,
                        )
                        for m in range(8):
                            for k in range(KH):
                                nc.tensor.matmul(
                                    ghA[:, m, :],
                                    whh_sb[:, k, m * P : (m + 1) * P],
                                    h_prev[:, k, :],
                                    start=False,
                                    stop=(m == 7 and k == KH - 1),
                                )
                        for m in range(8, 12):
                            for k in range(KH):
                                nc.tensor.matmul(
                                    ghB[:, m - 8, :],
                                    whh_sb[:, k, m * P : (m + 1) * P],
                                    h_prev[:, k, :],
                                    start=(m == 8 and k == 0),
                                    stop=False,
                                )
                        # b_hn for all 4 n-chunks in one K=4 matmul against an
                        # indicator rhs (bsel[k, m*BC+b] = k==m)
                        nc.tensor.matmul(
                            ghB[:].rearrange("p m b -> p (m b)"),
                            bhn_sb[:],
                            bsel_sb[:],
                            start=False,
                            stop=True,
                        )
                        # interleave next block's gx matmuls (keeps PE busy
                        # during the elementwise window)
                        if tb < NBLK - 1:
                            for g in gx_groups_due(ts):
                                emit_gx_group(tb + 1, g)

                        # ---- elementwise chain ----
                        rz = tmp_pool.tile([P, 8, BC], BF16, tag="rz", name="rz")
                        nc.scalar.activation(rz[:], ghA[:], AF.Sigmoid,
                                             scale=1.0 / WSCALE)
                        tn = tmp_pool.tile([P, KH, BC], BF16, tag="tn", name="tn")
                        nc.vector.scalar_tensor_tensor(
                            tn[:], ghB[:], 1.0 / WSCALE, rz[:, 0:KH, :],
                            OP.mult, OP.mult,
                        )
                        np_ = tmp_pool.tile([P, KH, BC], BF16, tag="np", name="np")
                        nc.vector.tensor_add(
                            np_[:], tn[:], gx_ring[:, 8:12, par, ts * BC : (ts + 1) * BC]
                        )
                        n_t = tmp_pool.tile([P, KH, BC], BF16, tag="nt", name="nt")
                        nc.scalar.activation(n_t[:], np_[:], AF.Tanh)
                        d1 = tmp_pool.tile([P, KH, BC], BF16, tag="d1", name="d1")
                        nc.vector.tensor_sub(d1[:], h_prev, n_t[:])
                        d2 = tmp_pool.tile([P, KH, BC], BF16, tag="d2", name="d2")
                        nc.vector.tensor_mul(d2[:], d1[:], rz[:, KH : 2 * KH, :])
                        nc.vector.tensor_add(
                            h_hist[:, :, par, ts * BC : (ts + 1) * BC], n_t[:], d2[:]
                        )
                    emit_flush(tb)

                # ---- exchange with partner core ----
                nc.gpsimd.collective_compute(
                    "AllGather",
                    OP.bypass,
                    replica_groups=[[c, c + 4] for c in range(4)],
                    ins=[rev_t.opt()],
                    outs=[gath_t.opt()],
                )
                gathered = gath_t

            # ---- final FC + sigmoid ----
            act_own_prev = act_a if n_layers % 2 == 1 else act_b
            fcw_sb = wpool.tile([P, 2 * KH, 1], BF16, tag="fcw")
            nc.sync.dma_start(fcw_sb[:], fcw_in.rearrange("(ko p) n -> p ko n", p=P))
            fcb_sb = wpool.tile([1, 1], F32, tag="fcb")
            nc.sync.dma_start(fcb_sb[:], fcb_in[:])
            for tb in range(NBLK):
                c0, c1 = tb * BLKC, (tb + 1) * BLKC
                st = stage_pool.tile([P, 2 * KH, BLKC], BF16, tag="stage")
                nc.sync.dma_start(
                    st[:, 0:KH, :],
                    act_own_prev[:, :, c0:c1].rearrange("k p c -> p k c"),
                )
                pr = pair_pool.tile([P, 2, KH, BLKC], BF16, tag="pr")
                nc.sync.dma_start(
                    pr[:], gathered[:, :, :, c0:c1].rearrange("j k p c -> p j k c")
                )
                dt_ = pair_pool.tile([P, KH, BLKC], BF16, tag="dsel")
                nc.vector.tensor_sub(dt_[:], pr[:, 0], pr[:, 1])
                nc.vector.scalar_tensor_tensor(
                    st[:, KH : 2 * KH, :], dt_[:], sel_sb[:, 0:1], pr[:, 1],
                    OP.mult, OP.add,
                )
                for half in range(2):
                    ps = gxps_pool.tile([1, BLKC // 2], F32, tag="gxps")
                    for k in range(2 * KH):
                        nc.tensor.matmul(
                            ps[:],
                            fcw_sb[:, k, :],
                            st[:, k, half * (BLKC // 2) : (half + 1) * (BLKC // 2)],
                            start=(k == 0),
                            stop=(k == 2 * KH - 1),
                        )
                    ob = tmp_pool.tile([1, BLKC // 2], F32, tag="ob")
                    nc.scalar.activation(ob[:], ps[:], AF.Sigmoid,
                                         bias=fcb_sb[:, 0:1])
                    nc.sync.dma_start(
                        out_d[0:1, c0 + half * (BLKC // 2) : c0 + (half + 1) * (BLKC // 2)],
                        ob[:],
                    )

    nc.finalize()
    return nc


def prep_inputs(input_seq, W_ih0, W_hh0, b_ih0, b_hh0, W_ih, W_hh, b_ih, b_hh,
                fc_w, fc_b, n_layers=L):
    """Host-side prep: per-core direction bake, transposes, bias folding."""
    bf = ml_dtypes.bfloat16
    whh_np = ml_dtypes.float8_e4m3fn if WHH_FP8 else bf

    x = np.asarray(input_seq)
    wih_all = [np.asarray(W_ih0)] + [np.asarray(W_ih)[l] for l in range(n_layers - 1)]
    whh_all = [np.asarray(W_hh0)] + [np.asarray(W_hh)[l] for l in range(n_layers - 1)]
    bih_all = [np.asarray(b_ih0)] + [np.asarray(b_ih)[l] for l in range(n_layers - 1)]
    bhh_all = [np.asarray(b_hh0)] + [np.asarray(b_hh)[l] for l in range(n_layers - 1)]

    ident = np.eye(P, dtype=bf)
    # bsel[k, m*BC+b] = 1 if k == m  (indicator rhs for the b_hn matmul)
    bsel = np.repeat(np.eye(KH, dtype=np.float32), BC, axis=1).astype(bf)
    in_maps = []
    for c in range(NCORES):
        d = 0 if c < 4 else 1
        shard = c % 4
        od = 1 - d

        whh = np.stack(
            [whh_all[l][d].T * WSCALE for l in range(n_layers)]
        ).astype(whh_np)
        wih0 = wih_all[0][d].T.astype(bf)  # [I, G3]
        wihs = []
        for l in range(1, n_layers):
            W = wih_all[l][d]  # [3H, 2H]; input feats [fwd H, bwd H]
            Wre = np.concatenate([W[:, d * H : (d + 1) * H],
                                  W[:, od * H : (od + 1) * H]], axis=1)
            wihs.append(Wre.T)  # [2H, G3]
        gxb = np.stack([bih_all[l][d].copy() for l in range(n_layers)])
        for l in range(n_layers):
            gxb[l][: 2 * H] += bhh_all[l][d][: 2 * H]
        gxb = np.ascontiguousarray(
            np.transpose(gxb.reshape(n_layers, M3, P), (0, 2, 1))
        ).astype(np.float32)
        bhnb = np.stack(
            [bhh_all[l][d][2 * H :].reshape(KH, P) * WSCALE
             for l in range(n_layers)]
        ).astype(bf)

        fw = np.asarray(fc_w)[0]  # [2H]
        fcw = np.concatenate([fw[d * H : (d + 1) * H], fw[od * H : (od + 1) * H]])

        xc = x[:, shard * BC : (shard + 1) * BC, :]
        if d == 1:
            xc = xc[::-1]
        xc = np.ascontiguousarray(xc.reshape(TOK, I).T)[None]

        m = {
            "x": xc.astype(bf),
            "wih0": np.ascontiguousarray(wih0),
            "whh": np.ascontiguousarray(whh),
            "gxb": gxb,
            "bhnb": np.ascontiguousarray(bhnb),
            "ident": ident,
            "bsel": bsel,
            "sel": np.full((P, 1), float(d), np.float32),
            "fcw": np.ascontiguousarray(fcw[:, None]).astype(bf),
            "fcb": np.asarray(fc_b, dtype=np.float32).reshape(1, 1),
        }
        if n_layers > 1:
            m["wih"] = np.ascontiguousarray(np.stack(wihs)).astype(bf)
        in_maps.append(m)
    return in_maps


def assemble_output(results):
    """Forward cores 0-3 hold the output for batch shards 0-3."""
    outs = []
    for c in range(4):
        o = np.asarray(results[c]["out"]).reshape(T, BC)
        outs.append(o)
    return np.concatenate(outs, axis=1)[:, :, None].astype(np.float32)


def kernel(**inputs):
    nc = build_bass()
    in_maps = prep_inputs(**inputs)
    res = run_bass_kernel_spmd(nc, in_maps, list(range(NCORES)))
    return assemble_output(res.results)
